# revision 7
# baseline (speedup 1.0000x reference)
"""BloomAttention (B=1, S=2048, HID=4096, NH=32) on 8 Trainium2 NeuronCores.

Strategy (tensor-parallel over heads, as the module does):
  - Each core owns 4 heads. w_qkv/b_qkv column-sharded (per-head q/k/v rows),
    INV_NORM folded into the q slice on host; weights shipped transposed+bf16.
  - On-device: hidden f32 -> bf16 cast + PE transpose -> hiddenT tiles;
    QKV matmul produces qT/kT/vT [d, s] per head directly.
  - Attention in transposed-scores layout: scoresT[sk, sq] = kT.T @ qT plus a
    2-row bias matmul adding (alibi_sk - alibi_sq); the per-query shift
    alibi_sq bounds exp() without a max-reduce (softmax is shift-invariant).
    Causal masking via additive -1e9 tiles on diagonal blocks. exp on ACT;
    P@V and the softmax denominator both as matmuls over the sk partitions.
  - AllToAll swaps head-shards for sequence-shards of the context, then each
    core computes its 256 output rows against the full (transposed, bf16)
    w_dense. Host just concatenates the 8 row-shards.
"""

import math
import os
import sys
import types
from contextlib import ExitStack

import numpy as np
import ml_dtypes

B, S, HID, NH, HD = 1, 2048, 4096, 32, 128
NCORES = 8
NH_LOC = NH // NCORES            # 4 heads per core
FQKV = NH_LOC * 3 * HD           # 1536 qkv features per core
SROW = S // NCORES               # 256 output rows per core
INV_NORM = 1.0 / math.sqrt(HD)
KT = HID // HD                   # 32 k tiles
KC = 16                          # k tiles cached in SBUF (rest streamed)
NEG = -1.0e9

_CACHE = {}


def _ensure_axon_hooks():
    try:
        import antenv  # noqa: F401

        extra = "/opt/trn_rl_repo/antenv"
        if os.path.isdir(extra) and extra not in antenv.__path__:
            antenv.__path__.append(extra)
        import antenv.axon_hooks  # noqa: F401
    except Exception:
        m = types.ModuleType("antenv.axon_hooks")
        m.get_axon_ntff_profile_hook = lambda: None
        m.set_axon_ntff_profile_hook = lambda h: None
        sys.modules["antenv.axon_hooks"] = m


def _build_nc():
    import concourse.bass as bass  # noqa: F401
    import concourse.mybir as mybir
    from concourse import bacc, tile
    from concourse.masks import make_identity

    BF = mybir.dt.bfloat16
    F32 = mybir.dt.float32
    Alu = mybir.AluOpType
    Act = mybir.ActivationFunctionType

    nc = bacc.Bacc(None, target_bir_lowering=False, num_devices=NCORES)
    with tile.TileContext(nc) as tc, ExitStack() as ctx:
        dram = ctx.enter_context(tc.tile_pool(name="dram", bufs=1, space="DRAM"))

        def din(name, shape, dt):
            return dram.tile(shape, dt, kind="ExternalInput", name=name,
                             uniquify=False)

        hidden = din("hidden", [S, HID], F32)
        wqkvT = din("wqkvT", [HID, FQKV], BF)
        bqkv = din("bqkv", [HD, NH_LOC * 3], F32)
        packL = din("packL", [2, NH_LOC * S], F32)
        packR = din("packR", [2, NH_LOC * S], F32)
        cmaskd = din("cmask", [HD, 4 * 512], F32)
        wdT = din("wdT", [HID, HID], BF)
        bdense = din("bdense", [1, HID], F32)
        out = dram.tile([SROW, HID], F32, kind="ExternalOutput", name="out",
                        uniquify=False)
        a2a_in = dram.tile([NCORES, NH_LOC, HD, SROW], BF, name="a2a_in")
        a2a_out = dram.tile([NCORES, NH_LOC, HD, SROW], BF, name="a2a_out")

        # ---------- persistent SBUF ----------
        const = ctx.enter_context(tc.tile_pool(name="const", bufs=1))
        sb_bqkv = const.tile([HD, NH_LOC * 3], F32)
        nc.sync.dma_start(out=sb_bqkv[:], in_=bqkv[:])
        ones_col = const.tile([HD, 1], BF)
        nc.vector.memset(ones_col[:], 1.0)
        ones_row = const.tile([1, HD], F32)
        nc.vector.memset(ones_row[:], 1.0)
        ident = const.tile([HD, HD], BF)
        make_identity(nc, ident[:])

        persist = ctx.enter_context(tc.tile_pool(name="persist", bufs=1))
        qT = [persist.tile([HD, S], BF, name=f"qT{h}") for h in range(NH_LOC)]
        kTt = [persist.tile([HD, S], BF, name=f"kT{h}") for h in range(NH_LOC)]
        fusedV = [persist.tile([HD, S], BF, name=f"fV{h}") for h in range(NH_LOC)]

        # ---------- phase 1: hiddenT + QKV ----------
        FG = [list(range(0, 6)), list(range(6, 12))]
        with (
            tc.tile_pool(name="wqc", bufs=1) as wqc_pool,
            tc.tile_pool(name="wstream", bufs=3) as ws_pool,
            tc.tile_pool(name="hT", bufs=1) as hT_pool,
            tc.tile_pool(name="stg", bufs=2) as stg_pool,
            tc.tile_pool(name="qkv_ps", bufs=1, space="PSUM") as qkv_ps,
            tc.tile_pool(name="tp_ps", bufs=2, space="PSUM") as tp_ps,
        ):
            wq_c = wqc_pool.tile([HD, KC, FQKV], BF)
            for kt in range(KC):
                nc.sync.dma_start(out=wq_c[:, kt, :],
                                  in_=wqkvT[kt * HD:(kt + 1) * HD, :])

            for sq in range(4):  # s-quarters of 512
                s0 = sq * 512
                hT_q = hT_pool.tile([HD, KT, 512], BF, name="hT_q")
                # cast + transpose this quarter of hidden
                for c2 in range(4):  # 128-row chunks
                    r0 = s0 + c2 * HD
                    for kh in range(2):
                        stg32 = stg_pool.tile([HD, 2048], F32, name="stg32")
                        nc.sync.dma_start(
                            out=stg32[:],
                            in_=hidden[r0:r0 + HD, kh * 2048:(kh + 1) * 2048])
                        stgbf = stg_pool.tile([HD, 2048], BF, name="stgbf")
                        nc.vector.tensor_copy(stgbf[:], stg32[:])
                        for k2 in range(16):
                            kt = kh * 16 + k2
                            pst = tp_ps.tile([HD, HD], BF, name="pst")
                            nc.tensor.transpose(
                                pst[:], stgbf[:, k2 * HD:(k2 + 1) * HD], ident[:])
                            nc.scalar.copy(
                                hT_q[:, kt, c2 * HD:(c2 + 1) * HD], pst[:])
                # QKV matmuls for this quarter
                for fg in FG:
                    nf = len(fg)
                    f0 = fg[0] * HD
                    psl = [qkv_ps.tile([HD, 512], F32, name=f"qkvps{i}",
                                       bufs=1) for i in range(nf)]
                    for kt in range(KT):
                        if kt < KC:
                            wsl = wq_c[:, kt, f0:f0 + nf * HD]
                        else:
                            ws = ws_pool.tile([HD, 6 * HD], BF, name="ws")
                            nc.sync.dma_start(
                                out=ws[:, :nf * HD],
                                in_=wqkvT[kt * HD:(kt + 1) * HD,
                                          f0:f0 + nf * HD])
                            wsl = ws[:, :nf * HD]
                        for i in range(nf):
                            nc.tensor.matmul(
                                psl[i][:],
                                wsl[:, i * HD:(i + 1) * HD],
                                hT_q[:, kt, :],
                                start=(kt == 0), stop=(kt == KT - 1))
                    for i, ft in enumerate(fg):
                        h, j = divmod(ft, 3)
                        dest = (qT, kTt, fusedV)[j][h]
                        nc.scalar.activation(
                            dest[:, s0:s0 + 512], psl[i][:], Act.Identity,
                            bias=sb_bqkv[:, ft:ft + 1])

        # ---------- phase 2: attention ----------
        with (
            tc.tile_pool(name="attn_sb", bufs=1) as attn_sb,
            tc.tile_pool(name="expp", bufs=3) as expp,
            tc.tile_pool(name="bcp", bufs=2) as bcp,
            tc.tile_pool(name="biasLp", bufs=2) as biasLp,
            tc.tile_pool(name="biasRp", bufs=2) as biasRp,
            tc.tile_pool(name="attn_ps", bufs=1, space="PSUM") as attn_ps,
            tc.tile_pool(name="sc_ps", bufs=2, space="PSUM") as sc_ps,
            tc.tile_pool(name="vt_ps", bufs=2, space="PSUM") as vt_ps,
        ):
            sb_cmask = attn_sb.tile([HD, 4 * 512], F32)
            nc.sync.dma_start(out=sb_cmask[:], in_=cmaskd[:])
            vnat = [attn_sb.tile([HD, S], BF, name=f"vn{h}")
                    for h in range(NH_LOC)]
            ctxT = [attn_sb.tile([HD, S], BF, name=f"cx{h}")
                    for h in range(NH_LOC)]
            for h in range(NH_LOC):
                for skt in range(16):
                    pv = vt_ps.tile([HD, HD], BF, name="pv")
                    nc.tensor.transpose(
                        pv[:], fusedV[h][:, skt * HD:(skt + 1) * HD], ident[:])
                    nc.scalar.copy(vnat[h][:, skt * HD:(skt + 1) * HD], pv[:])

            for h in range(NH_LOC):
                bL = biasLp.tile([2, S], F32, name="bL")
                nc.sync.dma_start(out=bL[:], in_=packL[:, h * S:(h + 1) * S])
                for sqb in range(4):
                    q0 = sqb * 512
                    nsk = 4 * (sqb + 1)
                    bR = biasRp.tile([2, 512], F32, name="bR")
                    nc.sync.dma_start(
                        out=bR[:], in_=packR[:, h * S + q0: h * S + q0 + 512])
                    ps_ctx = attn_ps.tile([HD, 512], F32, name="ps_ctx", bufs=2)
                    ps_sum = attn_ps.tile([1, 512], F32, name="ps_sum", bufs=1)
                    for skt in range(nsk):
                        ps = sc_ps.tile([HD, 512], F32, name="ps_sc")
                        nc.tensor.matmul(
                            ps[:], kTt[h][:, skt * HD:(skt + 1) * HD],
                            qT[h][:, q0:q0 + 512], start=True, stop=False)
                        nc.tensor.matmul(
                            ps[:], bL[:, skt * HD:(skt + 1) * HD], bR[:],
                            start=False, stop=True)
                        r = skt - 4 * sqb
                        if r >= 0:
                            nc.vector.tensor_tensor(
                                ps[:], ps[:], sb_cmask[:, r * 512:(r + 1) * 512],
                                Alu.add)
                        ex = expp.tile([HD, 512], BF, name="ex")
                        nc.scalar.activation(ex[:], ps[:], Act.Exp)
                        nc.tensor.matmul(
                            ps_ctx[:], vnat[h][:, skt * HD:(skt + 1) * HD],
                            ex[:], start=(skt == 0), stop=(skt == nsk - 1))
                        nc.tensor.matmul(
                            ps_sum[:], ones_col[:], ex[:],
                            start=(skt == 0), stop=(skt == nsk - 1))
                    recip = bcp.tile([1, 512], F32, name="recip")
                    nc.vector.reciprocal(recip[:], ps_sum[:])
                    ps_bc = attn_ps.tile([HD, 512], F32, name="ps_bc", bufs=1)
                    nc.tensor.matmul(ps_bc[:], ones_row[:], recip[:],
                                     start=True, stop=True)
                    bc = bcp.tile([HD, 512], F32, name="bc")
                    nc.scalar.copy(bc[:], ps_bc[:])
                    nc.vector.tensor_tensor(
                        ctxT[h][:, q0:q0 + 512], ps_ctx[:], bc[:], Alu.mult)

            # ---------- phase 3: all-to-all ----------
            for h in range(NH_LOC):
                for j in range(NCORES):
                    nc.sync.dma_start(out=a2a_in[j, h],
                                      in_=ctxT[h][:, j * SROW:(j + 1) * SROW])
            nc.gpsimd.collective_compute(
                "AllToAll", Alu.bypass,
                replica_groups=[list(range(NCORES))],
                ins=[a2a_in[:]], outs=[a2a_out[:]],
            )

        # ---------- phase 4: dense ----------
        with (
            tc.tile_pool(name="dns_sb", bufs=1) as dns_sb,
            tc.tile_pool(name="wd_pool", bufs=2) as wd_pool,
            tc.tile_pool(name="osb_pool", bufs=3) as osb_pool,
            tc.tile_pool(name="dns_ps", bufs=3, space="PSUM") as dns_ps,
        ):
            sb_bd = dns_sb.tile([1, HID], F32)
            nc.sync.dma_start(out=sb_bd[:], in_=bdense[:])
            crecv = dns_sb.tile([HD, KT, SROW], BF)
            for i in range(NCORES):
                for hl in range(NH_LOC):
                    nc.sync.dma_start(out=crecv[:, i * NH_LOC + hl, :],
                                      in_=a2a_out[i, hl])
            for ot in range(8):
                o0 = ot * 512
                wd = wd_pool.tile([HD, KT, 512], BF, name="wd")
                for ft in range(KT):
                    nc.sync.dma_start(
                        out=wd[:, ft, :],
                        in_=wdT[ft * HD:(ft + 1) * HD, o0:o0 + 512])
                for st in range(2):
                    psd = dns_ps.tile([HD, 512], F32, name="psd")
                    for ft in range(KT):
                        nc.tensor.matmul(
                            psd[:], crecv[:, ft, st * HD:(st + 1) * HD],
                            wd[:, ft, :], start=(ft == 0), stop=False)
                    nc.tensor.matmul(
                        psd[:], ones_row[:], sb_bd[:, o0:o0 + 512],
                        start=False, stop=True)
                    osb = osb_pool.tile([HD, 512], F32, name="osb")
                    nc.scalar.copy(osb[:], psd[:])
                    nc.sync.dma_start(
                        out=out[st * HD:(st + 1) * HD, o0:o0 + 512],
                        in_=osb[:])
    nc.compile()
    return nc


def _prep_shards(hidden_states, alibi, w_qkv, b_qkv, w_dense, b_dense):
    bf16 = ml_dtypes.bfloat16
    hidden = np.ascontiguousarray(
        np.asarray(hidden_states, dtype=np.float32).reshape(S, HID))
    al = np.asarray(alibi, dtype=np.float32).reshape(NH, S)
    w = np.asarray(w_qkv, dtype=np.float32)
    b = np.asarray(b_qkv, dtype=np.float32)
    wd = np.asarray(w_dense, dtype=np.float32)
    bd = np.asarray(b_dense, dtype=np.float32)

    # fold INV_NORM into the q projections
    scale = np.ones(3 * HID, np.float32)
    for h in range(NH):
        scale[h * 3 * HD:(h * 3 * HD) + HD] = INV_NORM
    wT = np.ascontiguousarray((w * scale[:, None]).T)      # [HID, 3*HID]
    bs = b * scale
    wdT = np.ascontiguousarray(wd.T).astype(bf16)          # [HID, HID]
    bdr = np.ascontiguousarray(bd.reshape(1, HID))

    # causal masks for the 4 diagonal 128x512 blocks
    a = np.arange(HD)[:, None]
    bq = np.arange(512)[None, :]
    cmask = np.concatenate(
        [np.where(r * HD + a <= bq, 0.0, NEG).astype(np.float32)
         for r in range(4)], axis=1)                        # [128, 4*512]

    in_maps = []
    for c in range(NCORES):
        f0 = c * FQKV
        heads = range(c * NH_LOC, (c + 1) * NH_LOC)
        alc = al[list(heads)]                               # [4, S]
        pl = np.concatenate([np.stack([np.ones(S, np.float32), alc[i]])
                             for i in range(NH_LOC)], axis=1)   # [2, 4*S]
        pr = np.concatenate([np.stack([-alc[i], np.ones(S, np.float32)])
                             for i in range(NH_LOC)], axis=1)   # [2, 4*S]
        in_maps.append({
            "hidden": hidden,
            "wqkvT": np.ascontiguousarray(wT[:, f0:f0 + FQKV]).astype(bf16),
            "bqkv": np.ascontiguousarray(
                bs[f0:f0 + FQKV].reshape(NH_LOC * 3, HD).T),
            "packL": pl,
            "packR": pr,
            "cmask": cmask,
            "wdT": wdT,
            "bdense": bdr,
        })
    return in_maps


def kernel(hidden_states, alibi, w_qkv, b_qkv, w_dense, b_dense):
    _ensure_axon_hooks()
    from concourse import bass_utils

    if "nc" not in _CACHE:
        _CACHE["nc"] = _build_nc()
    nc = _CACHE["nc"]
    in_maps = _prep_shards(hidden_states, alibi, w_qkv, b_qkv,
                           w_dense, b_dense)
    trace = bool(os.environ.get("BLOOM_TRACE"))
    res = bass_utils.run_bass_kernel_spmd(
        nc, in_maps, core_ids=list(range(NCORES)), trace=trace)
    kernel._last_results = res
    kernel._last_exec_ns = res.exec_time_ns
    outp = np.concatenate([res.results[c]["out"] for c in range(NCORES)],
                          axis=0)
    return outp.reshape(B, S, HID).astype(np.float32)
